# revision 18
# baseline (speedup 1.0000x reference)
"""Multi-head attention (b=4, n=2048, d=1024, h=16, dh=64) on 8 TRN2 NeuronCores.

Sharding: batch x sequence-half per core (core c handles batch b=c//2, query
rows s=(c%2)*1024 .. +1024). Each core recomputes K/V for its whole batch
locally (no collectives), computes flash-style attention for its 1024 query
rows over all 16 heads, applies the output projection, and writes a disjoint
1024-row slice of the flattened output.

Precision scheme (fp8 DoubleRow with residual correction):
  - Projections run in fp8e4m3 DoubleRow mode. Host ships x and W.T*32 split
    into (hi, lo) fp8 pairs; the kernel computes w8'x8 + w8'dx + dw'x8 over
    chunk-paired k-tiles (12 DR matmuls per 512-wide psum instead of 8 bf16
    matmuls -> 0.75x PE cycles). The dropped dw'dx term is ~1e-3 relative.
  - Scores likewise: Q/K are re-split on device into (q8,dq)/(k8,dk) during
    the PSUM->SBUF eviction. One DR matmul per (key-block, 512q) computes
    k8'q8 + k8'dq + dk'q8 via a stride-0 stationary k-tile trick -> 0.5x the
    bf16 score cost at better-than-bf16 accuracy.
  - exp/softmax, AV, and Wo stay bf16 (fp8 there fails the error budget).

Host-side layout choices (free transposes/permutes in numpy):
  x8/dx [d, 2048] = fp8 split of concat(x[b, my_half].T, x[b, other_half].T)
  wq8/dwq (etc) = fp8 split of W.T * 32 (scale keeps W out of fp8 subnormals;
      the PSUM eviction multiplies by 1/32, exactly, both powers of two).
  wot = Wo.T in bf16, bo as [1, d] f32.
"""

import sys

sys.path.insert(0, "/opt/trn_rl_repo")

from contextlib import ExitStack

import numpy as np

import concourse.bass as bass
import concourse.tile as tile
from concourse import bacc, mybir
from concourse.bass_utils import run_bass_kernel_spmd

F32 = mybir.dt.float32
BF16 = mybir.dt.bfloat16
F8 = mybir.dt.float8e4
U8 = mybir.dt.uint8
EXP = mybir.ActivationFunctionType.Exp
COPY = mybir.ActivationFunctionType.Copy
MUL = mybir.AluOpType.mult
SUB = mybir.AluOpType.subtract
DR = mybir.MatmulPerfMode.DoubleRow
I16 = mybir.dt.int16
ADD = mybir.AluOpType.add
SCHR_PERIOD = 4
TRUNC = int(__import__("os").environ.get("KERNEL_TRUNC", "0"))  # every Nth es tile -> DVE fast-exp (Schraudolph)
SCHR_A = 0.125 * 1.4426950408889634 * 128.0
SCHR_B = 127.0 * 128.0 - 5.504 + 0.5

P = 128
D = 1024  # model dim
NI = 1024  # query rows per core
NJ = 2048  # key rows per core (full batch)
H = 16  # heads
DH = 64  # head dim
SCALE = DH**-0.5  # 0.125
NCORES = 8

NCC = D // P  # 8 contraction chunks
NDB = D // P  # 8 feature blocks
ISC = 1.0 / 32.0  # weight-scale compensation (exact power of two)


def _dr_proj(nc, ps, w8, dw, x8, dx, osl, msl):
    """12 DR matmuls accumulating (w8+dw)'(x8+dx) ~ sum of 3 products into
    ps[:, :]. w8/dw are [P, 8, D]-chunked stationary u8 tiles (osl slices the
    out-feature dim), x8/dx are [P, 8, NJ]-chunked moving u8 tiles (msl slices
    the moving free dim)."""
    first = True
    for wt, xt in ((w8, x8), (w8, dx), (dw, x8)):
        for i in range(4):
            nc.tensor.matmul(
                ps[:],
                wt[:, 2 * i : 2 * i + 2, osl].bitcast(F8),
                xt[:, 2 * i : 2 * i + 2, msl].bitcast(F8),
                start=first,
                stop=(wt is dw and i == 3),
                perf_mode=DR,
            )
            first = False


def _dr_proj_xstat(nc, ps, x8, dx, w8, dw, nsl, osl):
    """Same 3-product DR accumulation with x stationary (for V): out
    [128 n-rows, 512 d_out]."""
    first = True
    for xt, wt in ((x8, w8), (x8, dw), (dx, w8)):
        for i in range(4):
            nc.tensor.matmul(
                ps[:],
                xt[:, 2 * i : 2 * i + 2, nsl].bitcast(F8),
                wt[:, 2 * i : 2 * i + 2, osl].bitcast(F8),
                start=first,
                stop=(xt is dx and i == 3),
                perf_mode=DR,
            )
            first = False


def _build():
    nc = bacc.Bacc("TRN2", target_bir_lowering=False, debug=False, num_devices=NCORES)

    x8d = nc.dram_tensor("x8", [D, NJ], U8, kind="ExternalInput").ap()
    dxd = nc.dram_tensor("dx", [D, NJ], U8, kind="ExternalInput").ap()
    wq8d = nc.dram_tensor("wq8", [D, D], U8, kind="ExternalInput").ap()
    dwqd = nc.dram_tensor("dwq", [D, D], U8, kind="ExternalInput").ap()
    wk8d = nc.dram_tensor("wk8", [D, D], U8, kind="ExternalInput").ap()
    dwkd = nc.dram_tensor("dwk", [D, D], U8, kind="ExternalInput").ap()
    wv8d = nc.dram_tensor("wv8", [D, D], U8, kind="ExternalInput").ap()
    dwvd = nc.dram_tensor("dwv", [D, D], U8, kind="ExternalInput").ap()
    wotd = nc.dram_tensor("wot", [D, D], BF16, kind="ExternalInput").ap()
    bod = nc.dram_tensor("bo", [1, D], F32, kind="ExternalInput").ap()
    out = nc.dram_tensor("out", [NI, D], BF16, kind="ExternalOutput").ap()

    with tile.TileContext(nc) as tc, ExitStack() as octx:
        # kernel-wide PSUM pools: 4 + 2 + 2 = 8 banks
        psA = octx.enter_context(tc.tile_pool(name="psA", bufs=2, space="PSUM"))
        psB = octx.enter_context(tc.tile_pool(name="psB", bufs=2, space="PSUM"))
        psC = octx.enter_context(tc.tile_pool(name="psC", bufs=2, space="PSUM"))
        dramp = octx.enter_context(tc.tile_pool(name="dramp", bufs=4, space="DRAM"))

        # persistent SBUF (KB/partition): QPACK 32, KPACK 32, V 33.25, X8 16,
        # DX 16, CTX 16, es 16, stg 1.5, os 2, bias 2  ~= 167
        xp = octx.enter_context(tc.tile_pool(name="xp", bufs=1))
        X8 = xp.tile([P, NCC, NJ], U8, name="X8")
        DX = xp.tile([P, NCC, NJ], U8, name="DX")
        kqp = octx.enter_context(tc.tile_pool(name="kqp", bufs=1))
        QPACK = kqp.tile([P, H, 2, NI], F8, name="QPACK")
        KPACK = kqp.tile([P, H, NJ], F8, name="KPACK")
        v_pool = octx.enter_context(tc.tile_pool(name="vp", bufs=1))
        vall = v_pool.tile([P, NJ // P, H, DH + 1], BF16, name="vall")
        V = [vall[:, j] for j in range(NJ // P)]

        ctx_pool = octx.enter_context(tc.tile_pool(name="ctxp", bufs=1, side="right"))
        CTX = [ctx_pool.tile([P, NI], BF16, tag=f"ctx{t}", name=f"ctx{t}") for t in range(NDB)]
        esp = octx.enter_context(tc.tile_pool(name="es", bufs=9))
        recp = octx.enter_context(tc.tile_pool(name="rec", bufs=6))
        stp = octx.enter_context(tc.tile_pool(name="stg", bufs=4))
        osp = octx.enter_context(tc.tile_pool(name="os", bufs=2))
        bip = octx.enter_context(tc.tile_pool(name="bias", bufs=1))

        wvhp = octx.enter_context(tc.tile_pool(name="wvh", bufs=1))

        def v_halfpass_jg(wv8t, dwvt, vh, jg):
            """Project V head-half vh for key group jg (4 n-blocks of 128)."""
            for j4 in range(4):
                j = jg * 4 + j4
                ps = psB.tile([P, 512], F32, tag="pj", name="pj")
                _dr_proj_xstat(
                    nc, ps, X8, DX, wv8t, dwvt,
                    slice(j * P, (j + 1) * P), slice(0, 512),
                )
                nc.vector.tensor_scalar_mul(
                    V[j][:, vh * 8 : (vh + 1) * 8, 0:DH],
                    ps[:].rearrange("p (h d) -> p h d", h=8),
                    ISC,
                )

        # ---------------- phase Q (DR fp8) -----------------------------------
        with tc.tile_pool(name="wq", bufs=1) as wqp:
            WQ8 = wqp.tile([P, NCC, D], U8, name="WQ8")
            DWQ = wqp.tile([P, NCC, D], U8, name="DWQ")
            # consolidated loads ordered by first use; sync queue carries the
            # critical prefix, gpsimd SWDGE carries the bulk.
            def chunked(dram, lo, hi):
                return dram[:, lo:hi].rearrange("(c p) n -> p c n", p=P)

            nc.sync.dma_start(WQ8[:, :, 0:2 * P], chunked(wq8d, 0, 2 * P))
            nc.sync.dma_start(X8[:, :, 0:512], chunked(x8d, 0, 512))
            nc.sync.dma_start(DWQ[:, :, 0:2 * P], chunked(dwqd, 0, 2 * P))
            nc.sync.dma_start(DX[:, :, 0:512], chunked(dxd, 0, 512))
            nc.sync.dma_start(WQ8[:, :, 2 * P : D], chunked(wq8d, 2 * P, D))
            nc.sync.dma_start(DWQ[:, :, 2 * P : D], chunked(dwqd, 2 * P, D))
            nc.gpsimd.dma_start(X8[:, :, 512:NJ], chunked(x8d, 512, NJ))
            nc.gpsimd.dma_start(DX[:, :, 512:NJ], chunked(dxd, 512, NJ))
            WV8a = wvhp.tile([P, NCC, 512], U8, tag="wv8", name="WV8a")
            DWVa = wvhp.tile([P, NCC, 512], U8, tag="dwv", name="DWVa")
            nc.gpsimd.dma_start(WV8a[:], wv8d[:, 0:512].rearrange("(c p) n -> p c n", p=P))
            nc.gpsimd.dma_start(DWVa[:], dwvd[:, 0:512].rearrange("(c p) n -> p c n", p=P))

            # QPACK zero slots (t=1, parts 64-127), one wide op on ACT
            nc.scalar.memzero(QPACK[64:128, :, 1, :])
            # V ones-columns, one strided op on DVE
            nc.vector.memset(vall[:, :, :, DH : DH + 1], 1.0)

            isl = slice(0, NI)
            for db in range(NDB):
                ps = psA.tile([P, NI], F32, tag="sp", name="qj")
                for ibh in range(2):
                    _dr_proj(
                        nc, ps[:, ibh * 512 : (ibh + 1) * 512],
                        WQ8, DWQ, X8, DX,
                        slice(db * P, (db + 1) * P),
                        slice(ibh * 512, (ibh + 1) * 512),
                    )
                hA, hB = 2 * db, 2 * db + 1
                # hi casts (ACT), residuals (DVE), dup (Pool)
                nc.scalar.activation(QPACK[0:64, hA, 0, isl], ps[0:64, :], COPY, scale=ISC)
                nc.scalar.activation(QPACK[0:64, hB, 0, isl], ps[64:128, :], COPY, scale=ISC)
                nc.vector.scalar_tensor_tensor(
                    QPACK[0:64, hA, 1, isl], ps[0:64, :], ISC, QPACK[0:64, hA, 0, isl], MUL, SUB
                )
                nc.vector.scalar_tensor_tensor(
                    QPACK[0:64, hB, 1, isl], ps[64:128, :], ISC, QPACK[0:64, hB, 0, isl], MUL, SUB
                )
                nc.gpsimd.tensor_copy(QPACK[64:128, hA, 0, isl], QPACK[0:64, hA, 0, isl])
                nc.gpsimd.tensor_copy(QPACK[64:128, hB, 0, isl], QPACK[0:64, hB, 0, isl])
                # V head-half 0 interleaves on psB while psA carries Q
                if db >= 4 and TRUNC != 1:
                    v_halfpass_jg(WV8a, DWVa, 0, db - 4)


        if TRUNC == 1:
            NDB_RUN = -1
        elif TRUNC in (2, 3):
            NDB_RUN = 1
        elif TRUNC == 4:
            NDB_RUN = 4
        else:
            NDB_RUN = NDB
        # Wo + K-proj weights load after the wq pool frees its SBUF
        wop = octx.enter_context(tc.tile_pool(name="wo", bufs=1))
        WO = [wop.tile([P, D], BF16, tag=f"wo{f}", name=f"wo{f}") for f in range(NCC)]
        wkp = octx.enter_context(tc.tile_pool(name="wk", bufs=1))
        WK8 = wkp.tile([P, NCC, D], U8, name="WK8")
        DWK = wkp.tile([P, NCC, D], U8, name="DWK")
        nc.gpsimd.dma_start(WK8[:, :, 0:2 * P], wk8d[:, 0:2 * P].rearrange("(c p) n -> p c n", p=P))
        nc.gpsimd.dma_start(DWK[:, :, 0:2 * P], dwkd[:, 0:2 * P].rearrange("(c p) n -> p c n", p=P))
        nc.gpsimd.dma_start(WK8[:, :, 2 * P : D], wk8d[:, 2 * P : D].rearrange("(c p) n -> p c n", p=P))
        nc.gpsimd.dma_start(DWK[:, :, 2 * P : D], dwkd[:, 2 * P : D].rearrange("(c p) n -> p c n", p=P))
        for f in range(NCC):
            nc.sync.dma_start(WO[f][:], wotd[f * P : (f + 1) * P, :])
        BIAS = bip.tile([P, D], BF16, name="BIAS")
        nc.gpsimd.dma_start(BIAS[:], bod.to_broadcast([P, D]))

        # -------- fused phase K + attention ---------------------------------
        if True:
            fillers = []
            es_ctr = [0]

            WV8b = DWVb = None
            for db in range(max(NDB_RUN, 0)):
                # V head-half 1 (heads 8-15): one key group per db in 1..4
                if db == 1:
                    WV8b = wvhp.tile([P, NCC, 512], U8, tag="wv8", name="WV8b")
                    DWVb = wvhp.tile([P, NCC, 512], U8, tag="dwv", name="DWVb")
                    nc.gpsimd.dma_start(WV8b[:], wv8d[:, 512:1024].rearrange("(c p) n -> p c n", p=P))
                    nc.gpsimd.dma_start(DWVb[:], dwvd[:, 512:1024].rearrange("(c p) n -> p c n", p=P))
                if 1 <= db <= 4:
                    v_halfpass_jg(WV8b, DWVb, 1, db - 1)
                # K projection for this db + KPACK split
                hA, hB = 2 * db, 2 * db + 1
                for jb in range(NJ // 512):
                    jsl = slice(jb * 512, (jb + 1) * 512)
                    ps = psB.tile([P, 512], F32, tag="pj", name="pj")
                    _dr_proj(nc, ps, WK8, DWK, X8, DX, slice(db * P, (db + 1) * P), jsl)
                    nc.vector.tensor_scalar_mul(KPACK[0:64, hA, jsl], ps[0:64, :], ISC)
                    nc.vector.tensor_scalar_mul(KPACK[0:64, hB, jsl], ps[64:128, :], ISC)
                    nc.vector.scalar_tensor_tensor(
                        KPACK[64:128, hA, jsl], ps[0:64, :], ISC, KPACK[0:64, hA, jsl], MUL, SUB
                    )
                    nc.vector.scalar_tensor_tensor(
                        KPACK[64:128, hB, jsl], ps[64:128, :], ISC, KPACK[0:64, hB, jsl], MUL, SUB
                    )
                # the two heads living in KPACK[:, 2db:2db+2], both i-blocks
                t = db
                for ib in range(0 if TRUNC == 2 else NI // 512):
                    if db == 7 and ib == 1:
                        fillers += [
                            lambda ib8=ib8, eb=eb: wo_final(ib8, eb)
                            for ib8 in range(4)
                            for eb in range(2)
                        ]
                    islc = slice(ib * 512, (ib + 1) * 512)
                    stgs = {}
                    for hh in range(2):
                        h = 2 * db + hh
                        dp = hh * DH
                        qmov = QPACK[:, h, :, islc]  # [128, 2, 512]
                        es_list = []
                        for pr in range(NJ // 256):
                            sp = psA.tile([P, 1024], F32, tag="sp", name="sp")
                            for half2 in range(2):
                                j = pr * 2 + half2
                                stat = (
                                    KPACK[:, h, j * P : (j + 1) * P]
                                    .rearrange("p (one m) -> p one m", one=1)
                                    .broadcast_to([P, 2, P])
                                )
                                nc.tensor.matmul(
                                    sp[:, half2 * 512 : (half2 + 1) * 512],
                                    stat,
                                    qmov,
                                    start=True,
                                    stop=True,
                                    perf_mode=DR,
                                )
                            es = esp.tile([P, 1024], BF16, tag="es", name="es")
                            es_ctr[0] += 1
                            if pr == 3 or (pr == 6 and hh == 0 and db % 2 == 1):
                                nc.vector.tensor_scalar(
                                    es[:].bitcast(I16), sp[:], SCHR_A, SCHR_B,
                                    op0=MUL, op1=ADD,
                                )
                            else:
                                nc.scalar.activation(es[:], sp[:], EXP, scale=SCALE)
                            es_list.append(es)
                            if fillers:
                                fillers.pop(0)()
                        # AV with es STATIONARY and V|1 MOVING (bf16).
                        # j-outer so each es tile is consumed as it lands; all
                        # four q-accumulators live in one 1-bank psum tile
                        # (start once zeroes the whole 2KB region).
                        ctp = psC.tile([P, 4, DH + 1], F32, tag="ct", name="ct")
                        nj = NJ // P
                        for j in range(nj):
                            for q in range(4):
                                nc.tensor.matmul(
                                    ctp[:, q, :],
                                    es_list[j // 2][
                                        :,
                                        (j % 2) * 512 + q * P : (j % 2) * 512 + (q + 1) * P,
                                    ],
                                    V[j][:, h, :],
                                    start=(j == 0 and q == 0),
                                    stop=(j == nj - 1 and q == 3),
                                    skip_group_check=True,
                                )
                        for q in range(4):
                            rec = recp.tile([P, 1], F32, tag="rec", name="rec")
                            nc.vector.reciprocal(rec[:], ctp[:, q, DH : DH + 1])
                            if hh == 0:
                                stgs[q] = stp.tile([P, 2 * DH], BF16, tag="st", name="st")
                            stg = stgs[q]
                            nc.vector.tensor_scalar_mul(
                                stg[:, dp : dp + DH], ctp[:, q, 0:DH], rec[:]
                            )
                            if hh == 1:
                                nc.sync.dma_start_transpose(
                                    CTX[t][:, ib * 512 + q * P : ib * 512 + (q + 1) * P],
                                    stg[:],
                                )

                # Wo partials park in dead KPACK heads, viewed as bf16:
                # park(ib8, eb) = KPACK head ib8, bytes [eb*1024:(eb+1)*1024]
                def park(ib8, eb):
                    return KPACK[:, ib8, eb * 1024 : (eb + 1) * 1024].bitcast(BF16)

                def wo_final(ib8, eb):
                    ps = psB.tile([P, 512], F32, tag="pj", name="pj")
                    nc.tensor.matmul(
                        ps[:],
                        CTX[7][:, ib8 * P : (ib8 + 1) * P],
                        WO[7][:, eb * 512 : (eb + 1) * 512],
                        start=True,
                        stop=True,
                    )
                    ostage = osp.tile([P, 512], BF16, tag="os", name="os")
                    nc.vector.tensor_add(ostage[:], ps[:], park(ib8, eb))
                    nc.sync.dma_start(
                        out[ib8 * P : (ib8 + 1) * P, eb * 512 : (eb + 1) * 512],
                        ostage[:],
                    )

                if db == 6:
                    def wo_part(ib8, eb):
                        ps = psB.tile([P, 512], F32, tag="pj", name="pj")
                        for f in range(7):
                            nc.tensor.matmul(
                                ps[:],
                                CTX[f][:, ib8 * P : (ib8 + 1) * P],
                                WO[f][:, eb * 512 : (eb + 1) * 512],
                                start=(f == 0),
                                stop=(f == 6),
                            )
                        nc.vector.tensor_add(
                            park(ib8, eb),
                            ps[:],
                            BIAS[:, eb * 512 : (eb + 1) * 512],
                        )

                    fillers += [
                        lambda ib8=ib8, eb=eb: wo_part(ib8, eb)
                        for ib8 in range(8)
                        for eb in range(2)
                    ]

            # ------------- phase Wo, final pass (rest) -----------------------
            for ib8 in range(4, NI // P) if TRUNC == 0 else ():
                for eb in range(2):
                    wo_final(ib8, eb)

    nc.compile()
    return nc


_NC = None


def _get_nc():
    global _NC
    if _NC is None:
        _NC = _build()
    return _NC


def _make_in_maps(x, Wq, Wk, Wv, Wo, bo):
    import ml_dtypes

    bf16 = ml_dtypes.bfloat16
    E4M3 = ml_dtypes.float8_e4m3
    f = np.float32

    def split8(a):
        hi = a.astype(E4M3)
        lo = (a - hi.astype(f)).astype(E4M3)
        return hi.view(np.uint8), lo.view(np.uint8)

    wq8, dwq = split8(np.ascontiguousarray(Wq.T).astype(f) * 32.0)
    wk8, dwk = split8(np.ascontiguousarray(Wk.T).astype(f) * 32.0)
    wv8, dwv = split8(np.ascontiguousarray(Wv.T).astype(f) * 32.0)
    wot = np.ascontiguousarray(Wo.T).astype(bf16)
    bo2 = np.ascontiguousarray(bo.reshape(1, D)).astype(np.float32)
    in_maps = []
    for c in range(NCORES):
        b, s = c // 2, c % 2
        mine = x[b, s * NI : (s + 1) * NI, :].T
        other = x[b, (1 - s) * NI : (2 - s) * NI, :].T
        xt = np.ascontiguousarray(np.concatenate([mine, other], axis=1)).astype(f)
        x8, dx = split8(xt)
        in_maps.append(
            {
                "x8": np.ascontiguousarray(x8),
                "dx": np.ascontiguousarray(dx),
                "wq8": wq8, "dwq": dwq,
                "wk8": wk8, "dwk": dwk,
                "wv8": wv8, "dwv": dwv,
                "wot": wot, "bo": bo2,
            }
        )
    return in_maps


def _run(x, Wq, Wk, Wv, Wo, bo, **spmd_kwargs):
    nc = _get_nc()
    in_maps = _make_in_maps(x, Wq, Wk, Wv, Wo, bo)
    res = run_bass_kernel_spmd(nc, in_maps, list(range(NCORES)), **spmd_kwargs)
    outs = [np.asarray(res.results[c]["out"]) for c in range(NCORES)]
    full = np.concatenate(outs, axis=0).reshape(4, 2048, D).astype(np.float32)
    return full, res


def kernel(x, Wq, Wk, Wv, Wo, bo):
    full, _ = _run(
        np.asarray(x), np.asarray(Wq), np.asarray(Wk), np.asarray(Wv),
        np.asarray(Wo), np.asarray(bo),
    )
    return full


# revision 19
# speedup vs baseline: 1.0116x; 1.0116x over previous
"""Multi-head attention (b=4, n=2048, d=1024, h=16, dh=64) on 8 TRN2 NeuronCores.

Sharding: batch x sequence-half per core (core c handles batch b=c//2, query
rows s=(c%2)*1024 .. +1024). Each core recomputes K/V for its whole batch
locally (no collectives), computes flash-style attention for its 1024 query
rows over all 16 heads, applies the output projection, and writes a disjoint
1024-row slice of the flattened output.

Precision scheme (fp8 DoubleRow with residual correction):
  - Projections run in fp8e4m3 DoubleRow mode. Host ships x and W.T*32 split
    into (hi, lo) fp8 pairs; the kernel computes w8'x8 + w8'dx + dw'x8 over
    chunk-paired k-tiles (12 DR matmuls per 512-wide psum instead of 8 bf16
    matmuls -> 0.75x PE cycles). The dropped dw'dx term is ~1e-3 relative.
  - Scores likewise: Q/K are re-split on device into (q8,dq)/(k8,dk) during
    the PSUM->SBUF eviction. One DR matmul per (key-block, 512q) computes
    k8'q8 + k8'dq + dk'q8 via a stride-0 stationary k-tile trick -> 0.5x the
    bf16 score cost at better-than-bf16 accuracy.
  - exp/softmax, AV, and Wo stay bf16 (fp8 there fails the error budget).

Host-side layout choices (free transposes/permutes in numpy):
  x8/dx [d, 2048] = fp8 split of concat(x[b, my_half].T, x[b, other_half].T)
  wq8/dwq (etc) = fp8 split of W.T * 32 (scale keeps W out of fp8 subnormals;
      the PSUM eviction multiplies by 1/32, exactly, both powers of two).
  wot = Wo.T in bf16, bo as [1, d] f32.
"""

import sys

sys.path.insert(0, "/opt/trn_rl_repo")

from contextlib import ExitStack

import numpy as np

import concourse.bass as bass
import concourse.tile as tile
from concourse import bacc, mybir
from concourse.bass_utils import run_bass_kernel_spmd

F32 = mybir.dt.float32
BF16 = mybir.dt.bfloat16
F8 = mybir.dt.float8e4
U8 = mybir.dt.uint8
EXP = mybir.ActivationFunctionType.Exp
COPY = mybir.ActivationFunctionType.Copy
MUL = mybir.AluOpType.mult
SUB = mybir.AluOpType.subtract
DR = mybir.MatmulPerfMode.DoubleRow
I16 = mybir.dt.int16
ADD = mybir.AluOpType.add
SCHR_PERIOD = 4
TRUNC = int(__import__("os").environ.get("KERNEL_TRUNC", "0"))  # every Nth es tile -> DVE fast-exp (Schraudolph)
SCHR_A = 0.125 * 1.4426950408889634 * 128.0
SCHR_B = 127.0 * 128.0 - 5.504 + 0.5

P = 128
D = 1024  # model dim
NI = 1024  # query rows per core
NJ = 2048  # key rows per core (full batch)
H = 16  # heads
DH = 64  # head dim
SCALE = DH**-0.5  # 0.125
NCORES = 8

NCC = D // P  # 8 contraction chunks
NDB = D // P  # 8 feature blocks
ISC = 1.0 / 32.0  # weight-scale compensation (exact power of two)


def _dr_proj(nc, ps, w8, dw, x8, dx, osl, msl):
    """12 DR matmuls accumulating (w8+dw)'(x8+dx) ~ sum of 3 products into
    ps[:, :]. w8/dw are [P, 8, D]-chunked stationary u8 tiles (osl slices the
    out-feature dim), x8/dx are [P, 8, NJ]-chunked moving u8 tiles (msl slices
    the moving free dim)."""
    first = True
    for wt, xt in ((w8, x8), (w8, dx), (dw, x8)):
        for i in range(4):
            nc.tensor.matmul(
                ps[:],
                wt[:, 2 * i : 2 * i + 2, osl].bitcast(F8),
                xt[:, 2 * i : 2 * i + 2, msl].bitcast(F8),
                start=first,
                stop=(wt is dw and i == 3),
                perf_mode=DR,
            )
            first = False


def _dr_proj_xstat(nc, ps, x8, dx, w8, dw, nsl, osl):
    """Same 3-product DR accumulation with x stationary (for V): out
    [128 n-rows, 512 d_out]."""
    first = True
    for xt, wt in ((x8, w8), (x8, dw), (dx, w8)):
        for i in range(4):
            nc.tensor.matmul(
                ps[:],
                xt[:, 2 * i : 2 * i + 2, nsl].bitcast(F8),
                wt[:, 2 * i : 2 * i + 2, osl].bitcast(F8),
                start=first,
                stop=(xt is dx and i == 3),
                perf_mode=DR,
            )
            first = False


def _build():
    nc = bacc.Bacc("TRN2", target_bir_lowering=False, debug=False, num_devices=NCORES)

    x8d = nc.dram_tensor("x8", [D, NJ], U8, kind="ExternalInput").ap()
    dxd = nc.dram_tensor("dx", [D, NJ], U8, kind="ExternalInput").ap()
    wq8d = nc.dram_tensor("wq8", [D, D], U8, kind="ExternalInput").ap()
    dwqd = nc.dram_tensor("dwq", [D, D], U8, kind="ExternalInput").ap()
    wk8d = nc.dram_tensor("wk8", [D, D], U8, kind="ExternalInput").ap()
    dwkd = nc.dram_tensor("dwk", [D, D], U8, kind="ExternalInput").ap()
    wv8d = nc.dram_tensor("wv8", [D, D], U8, kind="ExternalInput").ap()
    dwvd = nc.dram_tensor("dwv", [D, D], U8, kind="ExternalInput").ap()
    wotd = nc.dram_tensor("wot", [D, D], BF16, kind="ExternalInput").ap()
    bod = nc.dram_tensor("bo", [1, D], F32, kind="ExternalInput").ap()
    out = nc.dram_tensor("out", [NI, D], BF16, kind="ExternalOutput").ap()

    with tile.TileContext(nc) as tc, ExitStack() as octx:
        # kernel-wide PSUM pools: 4 + 2 + 2 = 8 banks
        psA = octx.enter_context(tc.tile_pool(name="psA", bufs=2, space="PSUM"))
        psB = octx.enter_context(tc.tile_pool(name="psB", bufs=2, space="PSUM"))
        psC = octx.enter_context(tc.tile_pool(name="psC", bufs=2, space="PSUM"))
        dramp = octx.enter_context(tc.tile_pool(name="dramp", bufs=4, space="DRAM"))

        # persistent SBUF (KB/partition): QPACK 32, KPACK 32, V 33.25, X8 16,
        # DX 16, CTX 16, es 16, stg 1.5, os 2, bias 2  ~= 167
        xp = octx.enter_context(tc.tile_pool(name="xp", bufs=1))
        X8 = xp.tile([P, NCC, NJ], U8, name="X8")
        DX = xp.tile([P, NCC, NJ], U8, name="DX")
        kqp = octx.enter_context(tc.tile_pool(name="kqp", bufs=1))
        QPACK = kqp.tile([P, H, 2, NI], F8, name="QPACK")
        KPACK = kqp.tile([P, H, NJ], F8, name="KPACK")
        v_pool = octx.enter_context(tc.tile_pool(name="vp", bufs=1))
        vall = v_pool.tile([P, NJ // P, H, DH + 1], BF16, name="vall")
        V = [vall[:, j] for j in range(NJ // P)]

        ctx_pool = octx.enter_context(tc.tile_pool(name="ctxp", bufs=1, side="right"))
        CTX = [ctx_pool.tile([P, NI], BF16, tag=f"ctx{t}", name=f"ctx{t}") for t in range(NDB)]
        esp = octx.enter_context(tc.tile_pool(name="es", bufs=9))
        recp = octx.enter_context(tc.tile_pool(name="rec", bufs=6))
        stp = octx.enter_context(tc.tile_pool(name="stg", bufs=4))
        osp = octx.enter_context(tc.tile_pool(name="os", bufs=2))
        bip = octx.enter_context(tc.tile_pool(name="bias", bufs=1))

        wvhp = octx.enter_context(tc.tile_pool(name="wvh", bufs=1))

        def v_halfpass_jg(wv8t, dwvt, vh, jg):
            """Project V head-half vh for key group jg (4 n-blocks of 128)."""
            for j4 in range(4):
                j = jg * 4 + j4
                ps = psB.tile([P, 512], F32, tag="pj", name="pj")
                _dr_proj_xstat(
                    nc, ps, X8, DX, wv8t, dwvt,
                    slice(j * P, (j + 1) * P), slice(0, 512),
                )
                nc.vector.tensor_scalar_mul(
                    V[j][:, vh * 8 : (vh + 1) * 8, 0:DH],
                    ps[:].rearrange("p (h d) -> p h d", h=8),
                    ISC,
                )

        # ---------------- phase Q (DR fp8) -----------------------------------
        with tc.tile_pool(name="wq", bufs=1) as wqp:
            WQ8 = wqp.tile([P, NCC, D], U8, name="WQ8")
            DWQ = wqp.tile([P, NCC, D], U8, name="DWQ")
            # consolidated loads ordered by first use; sync queue carries the
            # critical prefix, gpsimd SWDGE carries the bulk.
            def chunked(dram, lo, hi):
                return dram[:, lo:hi].rearrange("(c p) n -> p c n", p=P)

            nc.sync.dma_start(WQ8[:, :, 0:2 * P], chunked(wq8d, 0, 2 * P))
            nc.sync.dma_start(X8[:, :, 0:512], chunked(x8d, 0, 512))
            nc.sync.dma_start(DWQ[:, :, 0:2 * P], chunked(dwqd, 0, 2 * P))
            nc.sync.dma_start(DX[:, :, 0:512], chunked(dxd, 0, 512))
            nc.sync.dma_start(WQ8[:, :, 2 * P : D], chunked(wq8d, 2 * P, D))
            nc.sync.dma_start(DWQ[:, :, 2 * P : D], chunked(dwqd, 2 * P, D))
            nc.gpsimd.dma_start(X8[:, :, 512:NJ], chunked(x8d, 512, NJ))
            nc.gpsimd.dma_start(DX[:, :, 512:NJ], chunked(dxd, 512, NJ))
            WV8a = wvhp.tile([P, NCC, 512], U8, tag="wv8", name="WV8a")
            DWVa = wvhp.tile([P, NCC, 512], U8, tag="dwv", name="DWVa")
            nc.gpsimd.dma_start(WV8a[:], wv8d[:, 0:512].rearrange("(c p) n -> p c n", p=P))
            nc.gpsimd.dma_start(DWVa[:], dwvd[:, 0:512].rearrange("(c p) n -> p c n", p=P))

            # QPACK zero slots (t=1, parts 64-127), one wide op on ACT
            nc.scalar.memzero(QPACK[64:128, :, 1, :])
            # V ones-columns, one strided op on DVE
            nc.vector.memset(vall[:, :, :, DH : DH + 1], 1.0)

            isl = slice(0, NI)
            for db in range(NDB):
                ps = psA.tile([P, NI], F32, tag="sp", name="qj")
                for ibh in range(2):
                    _dr_proj(
                        nc, ps[:, ibh * 512 : (ibh + 1) * 512],
                        WQ8, DWQ, X8, DX,
                        slice(db * P, (db + 1) * P),
                        slice(ibh * 512, (ibh + 1) * 512),
                    )
                hA, hB = 2 * db, 2 * db + 1
                # hi casts (ACT), residuals (DVE), dup (Pool)
                nc.scalar.activation(QPACK[0:64, hA, 0, isl], ps[0:64, :], COPY, scale=ISC)
                nc.scalar.activation(QPACK[0:64, hB, 0, isl], ps[64:128, :], COPY, scale=ISC)
                nc.vector.scalar_tensor_tensor(
                    QPACK[0:64, hA, 1, isl], ps[0:64, :], ISC, QPACK[0:64, hA, 0, isl], MUL, SUB
                )
                nc.vector.scalar_tensor_tensor(
                    QPACK[0:64, hB, 1, isl], ps[64:128, :], ISC, QPACK[0:64, hB, 0, isl], MUL, SUB
                )
                nc.gpsimd.tensor_copy(QPACK[64:128, hA, 0, isl], QPACK[0:64, hA, 0, isl])
                nc.gpsimd.tensor_copy(QPACK[64:128, hB, 0, isl], QPACK[0:64, hB, 0, isl])
                # V head-half 0 interleaves on psB while psA carries Q
                if db >= 4 and TRUNC != 1:
                    v_halfpass_jg(WV8a, DWVa, 0, db - 4)


        if TRUNC == 1:
            NDB_RUN = -1
        elif TRUNC in (2, 3):
            NDB_RUN = 1
        elif TRUNC == 4:
            NDB_RUN = 4
        else:
            NDB_RUN = NDB
        # Wo + K-proj weights load after the wq pool frees its SBUF
        wop = octx.enter_context(tc.tile_pool(name="wo", bufs=1))
        WO = [wop.tile([P, D], BF16, tag=f"wo{f}", name=f"wo{f}") for f in range(NCC)]
        wkp = octx.enter_context(tc.tile_pool(name="wk", bufs=1))
        WK8 = wkp.tile([P, NCC, D], U8, name="WK8")
        DWK = wkp.tile([P, NCC, D], U8, name="DWK")
        nc.gpsimd.dma_start(WK8[:, :, 0:2 * P], wk8d[:, 0:2 * P].rearrange("(c p) n -> p c n", p=P))
        nc.gpsimd.dma_start(DWK[:, :, 0:2 * P], dwkd[:, 0:2 * P].rearrange("(c p) n -> p c n", p=P))
        nc.gpsimd.dma_start(WK8[:, :, 2 * P : D], wk8d[:, 2 * P : D].rearrange("(c p) n -> p c n", p=P))
        nc.gpsimd.dma_start(DWK[:, :, 2 * P : D], dwkd[:, 2 * P : D].rearrange("(c p) n -> p c n", p=P))
        for f in range(NCC):
            nc.sync.dma_start(WO[f][:], wotd[f * P : (f + 1) * P, :])
        BIAS = bip.tile([P, D], BF16, name="BIAS")
        nc.gpsimd.dma_start(BIAS[:], bod.to_broadcast([P, D]))

        # -------- fused phase K + attention ---------------------------------
        if True:
            fillers = []
            es_ctr = [0]

            WV8b = DWVb = None
            for db in range(max(NDB_RUN, 0)):
                # V head-half 1 (heads 8-15): one key group per db in 1..4
                if db == 1:
                    WV8b = wvhp.tile([P, NCC, 512], U8, tag="wv8", name="WV8b")
                    DWVb = wvhp.tile([P, NCC, 512], U8, tag="dwv", name="DWVb")
                    nc.gpsimd.dma_start(WV8b[:], wv8d[:, 512:1024].rearrange("(c p) n -> p c n", p=P))
                    nc.gpsimd.dma_start(DWVb[:], dwvd[:, 512:1024].rearrange("(c p) n -> p c n", p=P))
                if 1 <= db <= 4:
                    v_halfpass_jg(WV8b, DWVb, 1, db - 1)
                # K projection for this db + KPACK split
                hA, hB = 2 * db, 2 * db + 1
                for jb in range(NJ // 512):
                    jsl = slice(jb * 512, (jb + 1) * 512)
                    ps = psB.tile([P, 512], F32, tag="pj", name="pj")
                    _dr_proj(nc, ps, WK8, DWK, X8, DX, slice(db * P, (db + 1) * P), jsl)
                    nc.vector.tensor_scalar_mul(KPACK[0:64, hA, jsl], ps[0:64, :], ISC)
                    nc.vector.tensor_scalar_mul(KPACK[0:64, hB, jsl], ps[64:128, :], ISC)
                    nc.vector.scalar_tensor_tensor(
                        KPACK[64:128, hA, jsl], ps[0:64, :], ISC, KPACK[0:64, hA, jsl], MUL, SUB
                    )
                    nc.vector.scalar_tensor_tensor(
                        KPACK[64:128, hB, jsl], ps[64:128, :], ISC, KPACK[0:64, hB, jsl], MUL, SUB
                    )
                # the two heads living in KPACK[:, 2db:2db+2], both i-blocks
                t = db
                for ib in range(0 if TRUNC == 2 else NI // 512):
                    if db == 7 and ib == 1:
                        fillers += [
                            lambda ib8=ib8, eb=eb: wo_final(ib8, eb)
                            for ib8 in range(4)
                            for eb in range(2)
                        ]
                    islc = slice(ib * 512, (ib + 1) * 512)
                    stgs = {}
                    for hh in range(2):
                        h = 2 * db + hh
                        dp = hh * DH
                        qmov = QPACK[:, h, :, islc]  # [128, 2, 512]
                        es_list = []
                        for pr in range(NJ // 256):
                            sp = psA.tile([P, 1024], F32, tag="sp", name="sp")
                            for half2 in range(2):
                                j = pr * 2 + half2
                                stat = (
                                    KPACK[:, h, j * P : (j + 1) * P]
                                    .rearrange("p (one m) -> p one m", one=1)
                                    .broadcast_to([P, 2, P])
                                )
                                nc.tensor.matmul(
                                    sp[:, half2 * 512 : (half2 + 1) * 512],
                                    stat,
                                    qmov,
                                    start=True,
                                    stop=True,
                                    perf_mode=DR,
                                )
                            es = esp.tile([P, 1024], BF16, tag="es", name="es")
                            es_ctr[0] += 1
                            if pr == 3 or (pr == 6 and hh == 0 and db % 2 == 1):
                                nc.vector.tensor_scalar(
                                    es[:].bitcast(I16), sp[:], SCHR_A, SCHR_B,
                                    op0=MUL, op1=ADD,
                                )
                            else:
                                nc.scalar.activation(es[:], sp[:], EXP, scale=SCALE)
                            es_list.append(es)
                            if fillers:
                                fillers.pop(0)()
                        # AV with es STATIONARY and V|1 MOVING (bf16).
                        # j-outer so each es tile is consumed as it lands; all
                        # four q-accumulators live in one 1-bank psum tile
                        # (start once zeroes the whole 2KB region).
                        ctp = psC.tile([P, 4, DH + 1], F32, tag="ct", name="ct")
                        nj = NJ // P
                        for j in range(nj):
                            for q in range(4):
                                nc.tensor.matmul(
                                    ctp[:, q, :],
                                    es_list[j // 2][
                                        :,
                                        (j % 2) * 512 + q * P : (j % 2) * 512 + (q + 1) * P,
                                    ],
                                    V[j][:, h, :],
                                    start=(j == 0 and q == 0),
                                    stop=(j == nj - 1 and q == 3),
                                    skip_group_check=True,
                                )
                        for q in range(4):
                            rec = recp.tile([P, 1], F32, tag="rec", name="rec")
                            nc.vector.reciprocal(rec[:], ctp[:, q, DH : DH + 1])
                            if hh == 0:
                                stgs[q] = stp.tile([P, 2 * DH], BF16, tag="st", name="st")
                            stg = stgs[q]
                            nc.vector.tensor_scalar_mul(
                                stg[:, dp : dp + DH], ctp[:, q, 0:DH], rec[:]
                            )
                            if hh == 1:
                                nc.sync.dma_start_transpose(
                                    CTX[t][:, ib * 512 + q * P : ib * 512 + (q + 1) * P],
                                    stg[:],
                                )

                # Wo partials park in dead KPACK heads, viewed as bf16:
                # park(ib8, eb) = KPACK head ib8, bytes [eb*1024:(eb+1)*1024]
                def park(ib8, eb):
                    return KPACK[:, ib8, eb * 1024 : (eb + 1) * 1024].bitcast(BF16)

                def wo_final(ib8, eb):
                    ps = psB.tile([P, 512], F32, tag="pj", name="pj")
                    nc.tensor.matmul(
                        ps[:],
                        CTX[7][:, ib8 * P : (ib8 + 1) * P],
                        WO[7][:, eb * 512 : (eb + 1) * 512],
                        start=True,
                        stop=True,
                    )
                    ostage = osp.tile([P, 512], BF16, tag="os", name="os")
                    nc.vector.tensor_add(ostage[:], ps[:], park(ib8, eb))
                    nc.sync.dma_start(
                        out[ib8 * P : (ib8 + 1) * P, eb * 512 : (eb + 1) * 512],
                        ostage[:],
                    )

                if db == 5:
                    def wo_rmw(ib8, eb, f):
                        ps = psB.tile([P, 512], F32, tag="pj", name="pj")
                        nc.tensor.matmul(
                            ps[:],
                            CTX[f][:, ib8 * P : (ib8 + 1) * P],
                            WO[f][:, eb * 512 : (eb + 1) * 512],
                            start=True,
                            stop=True,
                        )
                        dst = park(ib8, eb)
                        nc.vector.tensor_add(dst, ps[:], dst)

                    fillers += [
                        lambda ib8=ib8, eb=eb: wo_rmw(ib8, eb, 5)
                        for ib8 in range(5)
                        for eb in range(2)
                    ]
                if db == 6:
                    fillers += [
                        lambda ib8=ib8, eb=eb: wo_rmw(ib8, eb, 6)
                        for ib8 in range(6)
                        for eb in range(2)
                    ]
                if db == 5:
                    fillers += [
                        lambda eb=eb: wo_part(5, eb, 6) for eb in range(2)
                    ]
                if db in (4, 6):
                    lo, hi = (0, 5) if db == 4 else (6, 8)
                    nf = 5 if db == 4 else 7

                    def wo_part(ib8, eb, nf):
                        ps = psB.tile([P, 512], F32, tag="pj", name="pj")
                        for f in range(nf):
                            nc.tensor.matmul(
                                ps[:],
                                CTX[f][:, ib8 * P : (ib8 + 1) * P],
                                WO[f][:, eb * 512 : (eb + 1) * 512],
                                start=(f == 0),
                                stop=(f == nf - 1),
                            )
                        nc.vector.tensor_add(
                            park(ib8, eb),
                            ps[:],
                            BIAS[:, eb * 512 : (eb + 1) * 512],
                        )

                    fillers += [
                        lambda ib8=ib8, eb=eb, nf=nf: wo_part(ib8, eb, nf)
                        for ib8 in range(lo, hi)
                        for eb in range(2)
                    ]

            # ------------- phase Wo, final pass (rest) -----------------------
            for ib8 in range(4, NI // P) if TRUNC == 0 else ():
                for eb in range(2):
                    wo_final(ib8, eb)

    nc.compile()
    return nc


_NC = None


def _get_nc():
    global _NC
    if _NC is None:
        _NC = _build()
    return _NC


def _make_in_maps(x, Wq, Wk, Wv, Wo, bo):
    import ml_dtypes

    bf16 = ml_dtypes.bfloat16
    E4M3 = ml_dtypes.float8_e4m3
    f = np.float32

    def split8(a):
        hi = a.astype(E4M3)
        lo = (a - hi.astype(f)).astype(E4M3)
        return hi.view(np.uint8), lo.view(np.uint8)

    wq8, dwq = split8(np.ascontiguousarray(Wq.T).astype(f) * 32.0)
    wk8, dwk = split8(np.ascontiguousarray(Wk.T).astype(f) * 32.0)
    wv8, dwv = split8(np.ascontiguousarray(Wv.T).astype(f) * 32.0)
    wot = np.ascontiguousarray(Wo.T).astype(bf16)
    bo2 = np.ascontiguousarray(bo.reshape(1, D)).astype(np.float32)
    in_maps = []
    for c in range(NCORES):
        b, s = c // 2, c % 2
        mine = x[b, s * NI : (s + 1) * NI, :].T
        other = x[b, (1 - s) * NI : (2 - s) * NI, :].T
        xt = np.ascontiguousarray(np.concatenate([mine, other], axis=1)).astype(f)
        x8, dx = split8(xt)
        in_maps.append(
            {
                "x8": np.ascontiguousarray(x8),
                "dx": np.ascontiguousarray(dx),
                "wq8": wq8, "dwq": dwq,
                "wk8": wk8, "dwk": dwk,
                "wv8": wv8, "dwv": dwv,
                "wot": wot, "bo": bo2,
            }
        )
    return in_maps


def _run(x, Wq, Wk, Wv, Wo, bo, **spmd_kwargs):
    nc = _get_nc()
    in_maps = _make_in_maps(x, Wq, Wk, Wv, Wo, bo)
    res = run_bass_kernel_spmd(nc, in_maps, list(range(NCORES)), **spmd_kwargs)
    outs = [np.asarray(res.results[c]["out"]) for c in range(NCORES)]
    full = np.concatenate(outs, axis=0).reshape(4, 2048, D).astype(np.float32)
    return full, res


def kernel(x, Wq, Wk, Wv, Wo, bo):
    full, _ = _run(
        np.asarray(x), np.asarray(Wq), np.asarray(Wk), np.asarray(Wv),
        np.asarray(Wo), np.asarray(bo),
    )
    return full


# revision 20
# speedup vs baseline: 1.0184x; 1.0067x over previous
"""Multi-head attention (b=4, n=2048, d=1024, h=16, dh=64) on 8 TRN2 NeuronCores.

Sharding: batch x sequence-half per core (core c handles batch b=c//2, query
rows s=(c%2)*1024 .. +1024). Each core recomputes K/V for its whole batch
locally (no collectives), computes flash-style attention for its 1024 query
rows over all 16 heads, applies the output projection, and writes a disjoint
1024-row slice of the flattened output.

Precision scheme (fp8 DoubleRow with residual correction):
  - Projections run in fp8e4m3 DoubleRow mode. Host ships x and W.T*32 split
    into (hi, lo) fp8 pairs; the kernel computes w8'x8 + w8'dx + dw'x8 over
    chunk-paired k-tiles (12 DR matmuls per 512-wide psum instead of 8 bf16
    matmuls -> 0.75x PE cycles). The dropped dw'dx term is ~1e-3 relative.
  - Scores likewise: Q/K are re-split on device into (q8,dq)/(k8,dk) during
    the PSUM->SBUF eviction. One DR matmul per (key-block, 512q) computes
    k8'q8 + k8'dq + dk'q8 via a stride-0 stationary k-tile trick -> 0.5x the
    bf16 score cost at better-than-bf16 accuracy.
  - exp/softmax, AV, and Wo stay bf16 (fp8 there fails the error budget).

Host-side layout choices (free transposes/permutes in numpy):
  x8/dx [d, 2048] = fp8 split of concat(x[b, my_half].T, x[b, other_half].T)
  wq8/dwq (etc) = fp8 split of W.T * 32 (scale keeps W out of fp8 subnormals;
      the PSUM eviction multiplies by 1/32, exactly, both powers of two).
  wot = Wo.T in bf16, bo as [1, d] f32.
"""

import sys

sys.path.insert(0, "/opt/trn_rl_repo")

from contextlib import ExitStack

import numpy as np

import concourse.bass as bass
import concourse.tile as tile
from concourse import bacc, mybir
from concourse.bass_utils import run_bass_kernel_spmd

F32 = mybir.dt.float32
BF16 = mybir.dt.bfloat16
F8 = mybir.dt.float8e4
U8 = mybir.dt.uint8
EXP = mybir.ActivationFunctionType.Exp
COPY = mybir.ActivationFunctionType.Copy
MUL = mybir.AluOpType.mult
SUB = mybir.AluOpType.subtract
DR = mybir.MatmulPerfMode.DoubleRow
I16 = mybir.dt.int16
ADD = mybir.AluOpType.add
SCHR_PERIOD = 4
TRUNC = int(__import__("os").environ.get("KERNEL_TRUNC", "0"))  # every Nth es tile -> DVE fast-exp (Schraudolph)
SCHR_A = 0.125 * 1.4426950408889634 * 128.0
SCHR_B = 127.0 * 128.0 - 5.504 + 0.5

P = 128
D = 1024  # model dim
NI = 1024  # query rows per core
NJ = 2048  # key rows per core (full batch)
H = 16  # heads
DH = 64  # head dim
SCALE = DH**-0.5  # 0.125
NCORES = 8

NCC = D // P  # 8 contraction chunks
NDB = D // P  # 8 feature blocks
ISC = 1.0 / 32.0  # weight-scale compensation (exact power of two)


def _dr_proj(nc, ps, w8, dw, x8, dx, osl, msl):
    """12 DR matmuls accumulating (w8+dw)'(x8+dx) ~ sum of 3 products into
    ps[:, :]. w8/dw are [P, 8, D]-chunked stationary u8 tiles (osl slices the
    out-feature dim), x8/dx are [P, 8, NJ]-chunked moving u8 tiles (msl slices
    the moving free dim)."""
    first = True
    for wt, xt in ((w8, x8), (w8, dx), (dw, x8)):
        for i in range(4):
            nc.tensor.matmul(
                ps[:],
                wt[:, 2 * i : 2 * i + 2, osl].bitcast(F8),
                xt[:, 2 * i : 2 * i + 2, msl].bitcast(F8),
                start=first,
                stop=(wt is dw and i == 3),
                perf_mode=DR,
            )
            first = False


def _dr_proj_xstat(nc, ps, x8, dx, w8, dw, nsl, osl):
    """Same 3-product DR accumulation with x stationary (for V): out
    [128 n-rows, 512 d_out]."""
    first = True
    for xt, wt in ((x8, w8), (x8, dw), (dx, w8)):
        for i in range(4):
            nc.tensor.matmul(
                ps[:],
                xt[:, 2 * i : 2 * i + 2, nsl].bitcast(F8),
                wt[:, 2 * i : 2 * i + 2, osl].bitcast(F8),
                start=first,
                stop=(xt is dx and i == 3),
                perf_mode=DR,
            )
            first = False


def _build():
    nc = bacc.Bacc("TRN2", target_bir_lowering=False, debug=False, num_devices=NCORES)

    x8d = nc.dram_tensor("x8", [D, NJ], U8, kind="ExternalInput").ap()
    dxd = nc.dram_tensor("dx", [D, NJ], U8, kind="ExternalInput").ap()
    wq8d = nc.dram_tensor("wq8", [D, D], U8, kind="ExternalInput").ap()
    dwqd = nc.dram_tensor("dwq", [D, D], U8, kind="ExternalInput").ap()
    wk8d = nc.dram_tensor("wk8", [D, D], U8, kind="ExternalInput").ap()
    dwkd = nc.dram_tensor("dwk", [D, D], U8, kind="ExternalInput").ap()
    wv8d = nc.dram_tensor("wv8", [D, D], U8, kind="ExternalInput").ap()
    dwvd = nc.dram_tensor("dwv", [D, D], U8, kind="ExternalInput").ap()
    wotd = nc.dram_tensor("wot", [D, D], BF16, kind="ExternalInput").ap()
    bod = nc.dram_tensor("bo", [1, D], F32, kind="ExternalInput").ap()
    out = nc.dram_tensor("out", [NI, D], BF16, kind="ExternalOutput").ap()

    with tile.TileContext(nc) as tc, ExitStack() as octx:
        # kernel-wide PSUM pools: 4 + 2 + 2 = 8 banks
        psA = octx.enter_context(tc.tile_pool(name="psA", bufs=2, space="PSUM"))
        psB = octx.enter_context(tc.tile_pool(name="psB", bufs=2, space="PSUM"))
        psC = octx.enter_context(tc.tile_pool(name="psC", bufs=2, space="PSUM"))
        dramp = octx.enter_context(tc.tile_pool(name="dramp", bufs=4, space="DRAM"))

        # persistent SBUF (KB/partition): QPACK 32, KPACK 32, V 33.25, X8 16,
        # DX 16, CTX 16, es 16, stg 1.5, os 2, bias 2  ~= 167
        xp = octx.enter_context(tc.tile_pool(name="xp", bufs=1))
        X8 = xp.tile([P, NCC, NJ], U8, name="X8")
        DX = xp.tile([P, NCC, NJ], U8, name="DX")
        kqp = octx.enter_context(tc.tile_pool(name="kqp", bufs=1))
        QPACK = kqp.tile([P, H, 2, NI], F8, name="QPACK")
        KPACK = kqp.tile([P, H, NJ], F8, name="KPACK")
        v_pool = octx.enter_context(tc.tile_pool(name="vp", bufs=1))
        vall = v_pool.tile([P, NJ // P, H, DH + 1], BF16, name="vall")
        V = [vall[:, j] for j in range(NJ // P)]

        ctx_pool = octx.enter_context(tc.tile_pool(name="ctxp", bufs=1, side="right"))
        CTX = [ctx_pool.tile([P, NI], BF16, tag=f"ctx{t}", name=f"ctx{t}") for t in range(NDB)]
        esp = octx.enter_context(tc.tile_pool(name="es", bufs=9))
        recp = octx.enter_context(tc.tile_pool(name="rec", bufs=6))
        stp = octx.enter_context(tc.tile_pool(name="stg", bufs=4))
        osp = octx.enter_context(tc.tile_pool(name="os", bufs=2))
        bip = octx.enter_context(tc.tile_pool(name="bias", bufs=1))

        wvhp = octx.enter_context(tc.tile_pool(name="wvh", bufs=1))

        def v_halfpass_jg(wv8t, dwvt, vh, jg):
            """Project V head-half vh for key group jg (4 n-blocks of 128)."""
            for j4 in range(4):
                j = jg * 4 + j4
                ps = psB.tile([P, 512], F32, tag="pj", name="pj")
                _dr_proj_xstat(
                    nc, ps, X8, DX, wv8t, dwvt,
                    slice(j * P, (j + 1) * P), slice(0, 512),
                )
                nc.vector.tensor_scalar_mul(
                    V[j][:, vh * 8 : (vh + 1) * 8, 0:DH],
                    ps[:].rearrange("p (h d) -> p h d", h=8),
                    ISC,
                )

        # ---------------- phase Q (DR fp8) -----------------------------------
        with tc.tile_pool(name="wq", bufs=1) as wqp:
            WQ8 = wqp.tile([P, NCC, D], U8, name="WQ8")
            DWQ = wqp.tile([P, NCC, D], U8, name="DWQ")
            # consolidated loads ordered by first use; sync queue carries the
            # critical prefix, gpsimd SWDGE carries the bulk.
            def chunked(dram, lo, hi):
                return dram[:, lo:hi].rearrange("(c p) n -> p c n", p=P)

            nc.sync.dma_start(WQ8[:, :, 0:2 * P], chunked(wq8d, 0, 2 * P))
            nc.sync.dma_start(X8[:, :, 0:512], chunked(x8d, 0, 512))
            nc.sync.dma_start(DWQ[:, :, 0:2 * P], chunked(dwqd, 0, 2 * P))
            nc.sync.dma_start(DX[:, :, 0:512], chunked(dxd, 0, 512))
            nc.sync.dma_start(WQ8[:, :, 2 * P : D], chunked(wq8d, 2 * P, D))
            nc.sync.dma_start(DWQ[:, :, 2 * P : D], chunked(dwqd, 2 * P, D))
            nc.gpsimd.dma_start(X8[:, :, 512:NJ], chunked(x8d, 512, NJ))
            nc.gpsimd.dma_start(DX[:, :, 512:NJ], chunked(dxd, 512, NJ))
            WV8a = wvhp.tile([P, NCC, 512], U8, tag="wv8", name="WV8a")
            DWVa = wvhp.tile([P, NCC, 512], U8, tag="dwv", name="DWVa")
            nc.gpsimd.dma_start(WV8a[:], wv8d[:, 0:512].rearrange("(c p) n -> p c n", p=P))
            nc.gpsimd.dma_start(DWVa[:], dwvd[:, 0:512].rearrange("(c p) n -> p c n", p=P))

            # QPACK zero slots (t=1, parts 64-127), one wide op on ACT
            nc.scalar.memzero(QPACK[64:128, :, 1, :])
            # V ones-columns, one strided op on DVE
            nc.vector.memset(vall[:, :, :, DH : DH + 1], 1.0)

            isl = slice(0, NI)
            for db in range(NDB):
                ps = psA.tile([P, NI], F32, tag="sp", name="qj")
                for ibh in range(2):
                    _dr_proj(
                        nc, ps[:, ibh * 512 : (ibh + 1) * 512],
                        WQ8, DWQ, X8, DX,
                        slice(db * P, (db + 1) * P),
                        slice(ibh * 512, (ibh + 1) * 512),
                    )
                hA, hB = 2 * db, 2 * db + 1
                # hi casts (ACT), residuals (DVE), dup (Pool)
                nc.scalar.activation(QPACK[0:64, hA, 0, isl], ps[0:64, :], COPY, scale=ISC)
                nc.scalar.activation(QPACK[0:64, hB, 0, isl], ps[64:128, :], COPY, scale=ISC)
                nc.vector.scalar_tensor_tensor(
                    QPACK[0:64, hA, 1, isl], ps[0:64, :], ISC, QPACK[0:64, hA, 0, isl], MUL, SUB
                )
                nc.vector.scalar_tensor_tensor(
                    QPACK[0:64, hB, 1, isl], ps[64:128, :], ISC, QPACK[0:64, hB, 0, isl], MUL, SUB
                )
                nc.gpsimd.tensor_copy(QPACK[64:128, hA, 0, isl], QPACK[0:64, hA, 0, isl])
                nc.gpsimd.tensor_copy(QPACK[64:128, hB, 0, isl], QPACK[0:64, hB, 0, isl])
                # V head-half 0 interleaves on psB while psA carries Q
                if db >= 4 and TRUNC != 1:
                    v_halfpass_jg(WV8a, DWVa, 0, db - 4)


        if TRUNC == 1:
            NDB_RUN = -1
        elif TRUNC in (2, 3):
            NDB_RUN = 1
        elif TRUNC == 4:
            NDB_RUN = 4
        else:
            NDB_RUN = NDB
        # Wo + K-proj weights load after the wq pool frees its SBUF
        wop = octx.enter_context(tc.tile_pool(name="wo", bufs=1))
        WO = [wop.tile([P, D], BF16, tag=f"wo{f}", name=f"wo{f}") for f in range(NCC)]
        wkp = octx.enter_context(tc.tile_pool(name="wk", bufs=1))
        WK8 = wkp.tile([P, NCC, D], U8, name="WK8")
        DWK = wkp.tile([P, NCC, D], U8, name="DWK")
        nc.gpsimd.dma_start(WK8[:, :, 0:2 * P], wk8d[:, 0:2 * P].rearrange("(c p) n -> p c n", p=P))
        nc.gpsimd.dma_start(DWK[:, :, 0:2 * P], dwkd[:, 0:2 * P].rearrange("(c p) n -> p c n", p=P))
        nc.gpsimd.dma_start(WK8[:, :, 2 * P : D], wk8d[:, 2 * P : D].rearrange("(c p) n -> p c n", p=P))
        nc.gpsimd.dma_start(DWK[:, :, 2 * P : D], dwkd[:, 2 * P : D].rearrange("(c p) n -> p c n", p=P))
        for f in range(NCC):
            nc.sync.dma_start(WO[f][:], wotd[f * P : (f + 1) * P, :])
        BIAS = bip.tile([P, D], BF16, name="BIAS")
        nc.gpsimd.dma_start(BIAS[:], bod.to_broadcast([P, D]))

        # -------- fused phase K + attention ---------------------------------
        if True:
            fillers = []
            es_ctr = [0]

            WV8b = DWVb = None
            for db in range(max(NDB_RUN, 0)):
                # V head-half 1 (heads 8-15): one key group per db in 1..4
                if db == 1:
                    WV8b = wvhp.tile([P, NCC, 512], U8, tag="wv8", name="WV8b")
                    DWVb = wvhp.tile([P, NCC, 512], U8, tag="dwv", name="DWVb")
                    nc.gpsimd.dma_start(WV8b[:], wv8d[:, 512:1024].rearrange("(c p) n -> p c n", p=P))
                    nc.gpsimd.dma_start(DWVb[:], dwvd[:, 512:1024].rearrange("(c p) n -> p c n", p=P))
                if 1 <= db <= 4:
                    v_halfpass_jg(WV8b, DWVb, 1, db - 1)
                # K projection for this db + KPACK split
                hA, hB = 2 * db, 2 * db + 1
                for jb in range(NJ // 512):
                    jsl = slice(jb * 512, (jb + 1) * 512)
                    ps = psB.tile([P, 512], F32, tag="pj", name="pj")
                    _dr_proj(nc, ps, WK8, DWK, X8, DX, slice(db * P, (db + 1) * P), jsl)
                    nc.vector.tensor_scalar_mul(KPACK[0:64, hA, jsl], ps[0:64, :], ISC)
                    nc.vector.tensor_scalar_mul(KPACK[0:64, hB, jsl], ps[64:128, :], ISC)
                    nc.vector.scalar_tensor_tensor(
                        KPACK[64:128, hA, jsl], ps[0:64, :], ISC, KPACK[0:64, hA, jsl], MUL, SUB
                    )
                    nc.vector.scalar_tensor_tensor(
                        KPACK[64:128, hB, jsl], ps[64:128, :], ISC, KPACK[0:64, hB, jsl], MUL, SUB
                    )
                # the two heads living in KPACK[:, 2db:2db+2], both i-blocks
                t = db
                for ib in range(0 if TRUNC == 2 else NI // 512):
                    if db == 7 and ib == 1:
                        fillers += [
                            lambda ib8=ib8, eb=eb: wo_final(ib8, eb)
                            for ib8 in range(4)
                            for eb in range(2)
                        ]
                    islc = slice(ib * 512, (ib + 1) * 512)
                    stgs = {}
                    for hh in range(2):
                        h = 2 * db + hh
                        dp = hh * DH
                        qmov = QPACK[:, h, :, islc]  # [128, 2, 512]
                        es_list = []
                        for pr in range(NJ // 256):
                            sp = psA.tile([P, 1024], F32, tag="sp", name="sp")
                            for half2 in range(2):
                                j = pr * 2 + half2
                                stat = (
                                    KPACK[:, h, j * P : (j + 1) * P]
                                    .rearrange("p (one m) -> p one m", one=1)
                                    .broadcast_to([P, 2, P])
                                )
                                nc.tensor.matmul(
                                    sp[:, half2 * 512 : (half2 + 1) * 512],
                                    stat,
                                    qmov,
                                    start=True,
                                    stop=True,
                                    perf_mode=DR,
                                )
                            es = esp.tile([P, 1024], BF16, tag="es", name="es")
                            es_ctr[0] += 1
                            if pr == 3 or (pr == 6 and hh == 0):
                                nc.vector.tensor_scalar(
                                    es[:].bitcast(I16), sp[:], SCHR_A, SCHR_B,
                                    op0=MUL, op1=ADD,
                                )
                            else:
                                nc.scalar.activation(es[:], sp[:], EXP, scale=SCALE)
                            es_list.append(es)
                            if fillers:
                                fillers.pop(0)()
                        # AV with es STATIONARY and V|1 MOVING (bf16).
                        # j-outer so each es tile is consumed as it lands; all
                        # four q-accumulators live in one 1-bank psum tile
                        # (start once zeroes the whole 2KB region).
                        ctp = psC.tile([P, 4, DH + 1], F32, tag="ct", name="ct")
                        nj = NJ // P
                        for j in range(nj):
                            for q in range(4):
                                nc.tensor.matmul(
                                    ctp[:, q, :],
                                    es_list[j // 2][
                                        :,
                                        (j % 2) * 512 + q * P : (j % 2) * 512 + (q + 1) * P,
                                    ],
                                    V[j][:, h, :],
                                    start=(j == 0 and q == 0),
                                    stop=(j == nj - 1 and q == 3),
                                    skip_group_check=True,
                                )
                        for q in range(4):
                            rec = recp.tile([P, 1], F32, tag="rec", name="rec")
                            nc.vector.reciprocal(rec[:], ctp[:, q, DH : DH + 1])
                            if hh == 0:
                                stgs[q] = stp.tile([P, 2 * DH], BF16, tag="st", name="st")
                            stg = stgs[q]
                            nc.vector.tensor_scalar_mul(
                                stg[:, dp : dp + DH], ctp[:, q, 0:DH], rec[:]
                            )
                            if hh == 1:
                                nc.sync.dma_start_transpose(
                                    CTX[t][:, ib * 512 + q * P : ib * 512 + (q + 1) * P],
                                    stg[:],
                                )

                # Wo partials park in dead KPACK heads, viewed as bf16:
                # park(ib8, eb) = KPACK head ib8, bytes [eb*1024:(eb+1)*1024]
                def park(ib8, eb):
                    return KPACK[:, ib8, eb * 1024 : (eb + 1) * 1024].bitcast(BF16)

                def wo_final(ib8, eb):
                    ps = psB.tile([P, 512], F32, tag="pj", name="pj")
                    nc.tensor.matmul(
                        ps[:],
                        CTX[7][:, ib8 * P : (ib8 + 1) * P],
                        WO[7][:, eb * 512 : (eb + 1) * 512],
                        start=True,
                        stop=True,
                    )
                    ostage = osp.tile([P, 512], BF16, tag="os", name="os")
                    nc.vector.tensor_add(ostage[:], ps[:], park(ib8, eb))
                    nc.sync.dma_start(
                        out[ib8 * P : (ib8 + 1) * P, eb * 512 : (eb + 1) * 512],
                        ostage[:],
                    )

                if db == 5:
                    def wo_rmw(ib8, eb, f):
                        ps = psB.tile([P, 512], F32, tag="pj", name="pj")
                        nc.tensor.matmul(
                            ps[:],
                            CTX[f][:, ib8 * P : (ib8 + 1) * P],
                            WO[f][:, eb * 512 : (eb + 1) * 512],
                            start=True,
                            stop=True,
                        )
                        dst = park(ib8, eb)
                        nc.vector.tensor_add(dst, ps[:], dst)

                    fillers += [
                        lambda ib8=ib8, eb=eb: wo_rmw(ib8, eb, 5)
                        for ib8 in range(5)
                        for eb in range(2)
                    ]
                if db == 6:
                    fillers += [
                        lambda ib8=ib8, eb=eb: wo_rmw(ib8, eb, 6)
                        for ib8 in range(6)
                        for eb in range(2)
                    ]
                if db == 5:
                    fillers += [
                        lambda eb=eb: wo_part(5, eb, 6) for eb in range(2)
                    ]
                if db in (4, 6):
                    lo, hi = (0, 5) if db == 4 else (6, 8)
                    nf = 5 if db == 4 else 7

                    def wo_part(ib8, eb, nf):
                        ps = psB.tile([P, 512], F32, tag="pj", name="pj")
                        for f in range(nf):
                            nc.tensor.matmul(
                                ps[:],
                                CTX[f][:, ib8 * P : (ib8 + 1) * P],
                                WO[f][:, eb * 512 : (eb + 1) * 512],
                                start=(f == 0),
                                stop=(f == nf - 1),
                            )
                        nc.vector.tensor_add(
                            park(ib8, eb),
                            ps[:],
                            BIAS[:, eb * 512 : (eb + 1) * 512],
                        )

                    fillers += [
                        lambda ib8=ib8, eb=eb, nf=nf: wo_part(ib8, eb, nf)
                        for ib8 in range(lo, hi)
                        for eb in range(2)
                    ]

            # ------------- phase Wo, final pass (rest) -----------------------
            for ib8 in range(4, NI // P) if TRUNC == 0 else ():
                for eb in range(2):
                    wo_final(ib8, eb)

    nc.compile()
    return nc


_NC = None


def _get_nc():
    global _NC
    if _NC is None:
        _NC = _build()
    return _NC


def _make_in_maps(x, Wq, Wk, Wv, Wo, bo):
    import ml_dtypes

    bf16 = ml_dtypes.bfloat16
    E4M3 = ml_dtypes.float8_e4m3
    f = np.float32

    def split8(a):
        hi = a.astype(E4M3)
        lo = (a - hi.astype(f)).astype(E4M3)
        return hi.view(np.uint8), lo.view(np.uint8)

    wq8, dwq = split8(np.ascontiguousarray(Wq.T).astype(f) * 32.0)
    wk8, dwk = split8(np.ascontiguousarray(Wk.T).astype(f) * 32.0)
    wv8, dwv = split8(np.ascontiguousarray(Wv.T).astype(f) * 32.0)
    wot = np.ascontiguousarray(Wo.T).astype(bf16)
    bo2 = np.ascontiguousarray(bo.reshape(1, D)).astype(np.float32)
    in_maps = []
    for c in range(NCORES):
        b, s = c // 2, c % 2
        mine = x[b, s * NI : (s + 1) * NI, :].T
        other = x[b, (1 - s) * NI : (2 - s) * NI, :].T
        xt = np.ascontiguousarray(np.concatenate([mine, other], axis=1)).astype(f)
        x8, dx = split8(xt)
        in_maps.append(
            {
                "x8": np.ascontiguousarray(x8),
                "dx": np.ascontiguousarray(dx),
                "wq8": wq8, "dwq": dwq,
                "wk8": wk8, "dwk": dwk,
                "wv8": wv8, "dwv": dwv,
                "wot": wot, "bo": bo2,
            }
        )
    return in_maps


def _run(x, Wq, Wk, Wv, Wo, bo, **spmd_kwargs):
    nc = _get_nc()
    in_maps = _make_in_maps(x, Wq, Wk, Wv, Wo, bo)
    res = run_bass_kernel_spmd(nc, in_maps, list(range(NCORES)), **spmd_kwargs)
    outs = [np.asarray(res.results[c]["out"]) for c in range(NCORES)]
    full = np.concatenate(outs, axis=0).reshape(4, 2048, D).astype(np.float32)
    return full, res


def kernel(x, Wq, Wk, Wv, Wo, bo):
    full, _ = _run(
        np.asarray(x), np.asarray(Wq), np.asarray(Wk), np.asarray(Wv),
        np.asarray(Wo), np.asarray(bo),
    )
    return full
